# revision 12
# baseline (speedup 1.0000x reference)
"""MoE block (B=4,T=2048,D=2048,E=4,H=8192,TOPK=2,cap=2048) on 8 TRN2 NeuronCores.

Strategy:
  - Router + top-k + capacity selection on host (exact jax-on-CPU replication of
    the reference routing math, so routing decisions match bit-for-bit).
  - Expert-parallel device FFN: core c handles expert c//2, token half c%2.
    Each core computes yT = W2[e]^T @ gelu(W1[e]^T @ xT) for its 1024 tokens,
    all fp32 (fp32r matmuls on the PE at full 1 cycle/row rate).
  - Host combines: scale by router prob and scatter-add into the output.

Device kernel layout (per core, all fp32/fp32r; host pre-transposes so every
DMA line is >=2KB contiguous per partition):
  xT   [128, 16, 1024]     xT[p, kc, m]      = tok[m, kc*128+p]
  w1a  [128, 64, 16, 128]  w1a[p, hc, kc, h] = W1[e][kc*128+p, hc*128+h]
  w2s  [128, 16, 64, 128]  w2s[p, dc, hc, d] = W2[e][hc*128+p, dc*128+d]
  yT   [128, 16, 1024]     yT[p, dc, m]      = y[m, dc*128+p]
H is processed in 8 slices of 1024 (8 H-chunks of 128) so each weight byte
streams from HBM exactly once (DMA ~144MB/core vs PE ~930us — compute-bound).
Per slice: phase A computes hT for all 1024 tokens (PSUM K-accumulation over
16 D-chunks, exact Gelu on ACT into an f32r SBUF tile); phase B contracts the
slice into a persistent f32 SBUF accumulator y_sb via DVE adds from PSUM.
Matmuls are [128,128]x[128,512] fp32r, measured 227ns/MM steady-state (94% of
the PE streaming roofline); total ~962us/core HW time.
"""

import os

import numpy as np

B, T, D, E, H = 4, 2048, 2048, 4, 8192
TOPK = 2
N = B * T
CAP = N // E          # 2048 tokens per expert
M = CAP // 2          # 1024 tokens per core
KC = D // 128         # 16
HC = H // 128         # 64
TT = 512              # token tile (fp32 moving-operand max)
NT = M // TT          # 2 token tiles

_nc_cache = [None]


def _build_nc():
    import concourse.tile as tile
    import concourse.mybir as mybir
    from concourse import bacc
    from concourse.bass import ts

    F32 = mybir.dt.float32
    F32R = mybir.dt.float32r
    GELU = mybir.ActivationFunctionType.Gelu

    nc = bacc.Bacc(None, target_bir_lowering=False)
    xT = nc.declare_dram_parameter("xT", [128, KC, M], F32R, isOutput=False)
    w1a = nc.declare_dram_parameter("w1a", [128, HC, KC, 128], F32R, isOutput=False)
    w2s = nc.declare_dram_parameter("w2s", [128, KC, HC, 128], F32R, isOutput=False)
    yT = nc.declare_dram_parameter("yT", [128, KC, M], F32, isOutput=True)

    HQ = 8  # H-chunks (of 128) per pass
    NQ = HC // HQ  # 8 passes; weights stream exactly once

    with tile.TileContext(nc) as tc:
        with (
            tc.tile_pool(name="xpool", bufs=1) as xpool,
            tc.tile_pool(name="ypool", bufs=1) as ypool,
            tc.tile_pool(name="w1pool", bufs=4) as w1pool,
            tc.tile_pool(name="w2pool", bufs=3) as w2pool,
            tc.tile_pool(name="hpool", bufs=1) as hpool,
            tc.tile_pool(name="psa", bufs=4, space="PSUM") as psa,
            tc.tile_pool(name="psb", bufs=4, space="PSUM") as psb,
        ):
            x_sb = xpool.tile([128, KC, M], F32R, tag="x")
            # Warm the PE HAM clock (cold 1.2GHz -> warm 2.4GHz needs ~3.4us of
            # sustained PE activity) with junk matmuls on a zeroed tile while
            # the startup DMAs are still in flight. Costs nothing (PE would be
            # idle) and removes the ~3us cold-clock penalty on the first real
            # A-group.
            # Startup is DMA-paced: the first A-group needs w1[0] + x[t=0]
            # immediately (so they go first, x in 1MB chunks for subtile-dep
            # granularity), w1[i] every ~3.6us after, and x[t=1] only by
            # ~+29us — interleave its 1MB chunks between the early w1 loads
            # so the FIFO queue serves everything just in time.
            N_EARLY_W1 = 5
            w1_early = [
                w1pool.tile([128, KC, 128], F32R, tag="w1", name="w1_early0")
            ]
            for c in range(4):
                nc.sync.dma_start(w1_early[0][:, ts(c, 4)], w1a[:, 0, ts(c, 4)])
                nc.sync.dma_start(
                    x_sb[:, ts(c, 4), ts(0, TT)], xT[:, ts(c, 4), ts(0, TT)]
                )
            for i in range(1, N_EARLY_W1):
                w1_sb = w1pool.tile([128, KC, 128], F32R, tag="w1", name=f"w1_early{i}")
                nc.sync.dma_start(w1_sb[:], w1a[:, i])
                w1_early.append(w1_sb)
                # x[t=1] quarter-chunks: [128, 4, 512] = 1MB each
                nc.sync.dma_start(
                    x_sb[:, ts(i - 1, 4), ts(1, TT)], xT[:, ts(i - 1, 4), ts(1, TT)]
                )
            y_sb = ypool.tile([128, KC, M], F32, tag="y")

            for q in range(NQ):
                hT_sb = hpool.tile([128, HQ, M], F32R, tag="h")

                # Phase A: hT[q] = gelu(W1[:, q]^T @ x), all M tokens
                for i in range(HQ):
                    hc = q * HQ + i
                    if q == 0 and i < N_EARLY_W1:
                        w1_sb = w1_early[i]
                    else:
                        w1_sb = w1pool.tile([128, KC, 128], F32R, tag="w1")
                        nc.sync.dma_start(w1_sb[:], w1a[:, hc])
                    for t in range(NT):
                        ps = psa.tile([128, TT], F32, tag="psa")
                        for kc in range(KC):
                            nc.tensor.matmul(
                                ps[:],
                                lhsT=w1_sb[:, kc],
                                rhs=x_sb[:, kc, ts(t, TT)],
                                start=(kc == 0),
                                stop=(kc == KC - 1),
                            )
                        nc.scalar.activation(
                            hT_sb[:, i, ts(t, TT)], ps[:], GELU
                        )

                # Phase B: y += W2[q]^T @ hT[q]
                for dc in range(KC):
                    w2_sb = w2pool.tile([128, HQ, 128], F32R, tag="w2")
                    nc.sync.dma_start(w2_sb[:], w2s[:, dc, ts(q, HQ)])
                    for t in range(NT):
                        ps2 = psb.tile([128, TT], F32, tag="psb")
                        for i in range(HQ):
                            nc.tensor.matmul(
                                ps2[:],
                                lhsT=w2_sb[:, i],
                                rhs=hT_sb[:, i, ts(t, TT)],
                                start=(i == 0),
                                stop=(i == HQ - 1),
                            )
                        if q == 0:
                            nc.vector.tensor_copy(y_sb[:, dc, ts(t, TT)], ps2[:])
                        else:
                            nc.vector.tensor_add(
                                out=y_sb[:, dc, ts(t, TT)],
                                in0=y_sb[:, dc, ts(t, TT)],
                                in1=ps2[:],
                            )
                        if q == NQ - 1:
                            nc.sync.dma_start(
                                yT[:, dc, ts(t, TT)], y_sb[:, dc, ts(t, TT)]
                            )
    nc.finalize()
    return nc


def _route(x, Wg, bg):
    """Replicate the reference routing math with jax on CPU.

    Returns (sel_idx, p): [E, CAP] int32 token ids and [E, CAP] f32 weights.
    """
    import jax
    import jax.numpy as jnp

    cpu = jax.devices("cpu")[0]
    with jax.default_device(cpu):
        flat_x = jnp.asarray(x.reshape(N, D))
        logits = flat_x @ jnp.asarray(Wg) + jnp.asarray(bg)
        top_vals, top_idx = jax.lax.top_k(logits, TOPK)
        sparse = jnp.full_like(logits, -jnp.inf)
        sparse = sparse.at[jnp.arange(N)[:, None], top_idx].set(top_vals)
        probs = jax.nn.softmax(sparse, axis=-1)

        sel_idx = np.zeros((E, CAP), dtype=np.int64)
        p_all = np.zeros((E, CAP), dtype=np.float32)
        for i in range(E):
            assigned = (top_idx == i).any(axis=-1)
            score = jnp.where(assigned, probs[:, i], -jnp.inf)
            sel_p, sidx = jax.lax.top_k(score, CAP)
            p = jnp.where(jnp.isfinite(sel_p), sel_p, 0.0)
            sel_idx[i] = np.asarray(sidx)
            p_all[i] = np.asarray(p)
    return sel_idx, p_all


def kernel(x, Wg, bg, W1, W2):
    from concourse.bass_utils import run_bass_kernel_spmd

    x = np.asarray(x, dtype=np.float32)
    W1 = np.asarray(W1, dtype=np.float32)
    W2 = np.asarray(W2, dtype=np.float32)
    sel_idx, p_all = _route(x, np.asarray(Wg, np.float32), np.asarray(bg, np.float32))

    flat_x = x.reshape(N, D)

    # Host dispatch + weight shuffles.
    w1a = [
        np.ascontiguousarray(W1[e].reshape(KC, 128, HC, 128).transpose(1, 2, 0, 3))
        for e in range(E)
    ]
    w2s = [
        np.ascontiguousarray(W2[e].reshape(HC, 128, KC, 128).transpose(1, 2, 0, 3))
        for e in range(E)
    ]
    in_maps = []
    for c in range(8):
        e, half = divmod(c, 2)
        tok = flat_x[sel_idx[e, half * M : (half + 1) * M]]  # [M, D]
        xT = np.ascontiguousarray(tok.T.reshape(KC, 128, M).transpose(1, 0, 2))
        in_maps.append({"xT": xT, "w1a": w1a[e], "w2s": w2s[e]})

    if _nc_cache[0] is None:
        _nc_cache[0] = _build_nc()
    nc = _nc_cache[0]

    trace = bool(os.environ.get("BASS_MOE_TRACE"))
    kwargs = {}
    if trace:
        import sys
        import types

        try:
            from antenv.axon_hooks import get_axon_ntff_profile_hook  # noqa: F401
        except ImportError:
            from trn_agent_boot.trn_boot import _ntff_profile_via_ctypes

            hook = _ntff_profile_via_ctypes("/opt/axon/libaxon_pjrt.so")
            mod = types.ModuleType("antenv.axon_hooks")
            mod.get_axon_ntff_profile_hook = lambda: hook
            import antenv  # noqa: F401

            sys.modules["antenv.axon_hooks"] = mod
        tcores = [int(c) for c in os.environ.get("BASS_MOE_TRACE_CORES", "0").split(",")]
        kwargs = {"trace": True, "trace_cores": tcores}

    res = run_bass_kernel_spmd(nc, in_maps, core_ids=list(range(8)), **kwargs)
    if trace:
        kernel.last_exec_time_ns = res.exec_time_ns
        if res.exec_time_ns is not None:
            print(f"HW exec time: {res.exec_time_ns} ns")

    # Host combine: y = yT^T * p, scatter-add per expert in order.
    out = np.zeros((N, D), dtype=np.float32)
    for c in range(8):
        e, half = divmod(c, 2)
        yT = res.results[c]["yT"]  # [128, KC, M]
        y = yT.transpose(1, 0, 2).reshape(D, M).T  # [M, D]
        p = p_all[e, half * M : (half + 1) * M]
        np.add.at(out, sel_idx[e, half * M : (half + 1) * M], y * p[:, None])
    return out.reshape(B, T, D)
